# revision 15
# baseline (speedup 1.0000x reference)
"""MoE layer (top-2 routing, SwiGLU experts) for Trainium2, 8 NeuronCores.

Strategy: balanced expert parallelism. The gate (0.03% of FLOPs) and the
token dispatch/combine run on host. Tokens routed to each expert are
padded to 128-token tiles; the global tile count is split evenly across
the 8 cores (capacity = ceil(total_tiles/8) tiles/core instead of the
max-loaded expert), so every core does near-identical PE work. Each core
processes TWO token segments — slot A (kA tiles) and slot B (kB tiles)
with a GLOBAL static boundary — each segment belonging to one expert
whose weights are per-core inputs. Expert tile counts decompose into
kA/kB-sized pieces (e.g. 16=8+8, 17=9+8, 18=9+9), found by a small DP;
if no decomposition exists the kernel falls back to plain one-expert-
per-core with capacity = max expert count.

Each core runs the dense SwiGLU FFN in bf16 (PSUM accumulation is fp32;
tolerance is 2e-2, bf16 lands ~4e-3):

  phase 1:  h = silu(x @ w1) * (x @ w3)      (spilled to DRAM, token-tiled)
  phase 2:  y = (h @ w2) * route_weight      (tokens on partitions)

x is passed transposed ([H, CAP]) so phase 1 streams with tokens on the
free dimension and phase 2 uses h (token columns) as the matmul
stationary — no on-device transposes. w1/w3 are passed pre-blocked
[P, IB, HB, P] so every DMA moves >=1KB-contiguous per-partition
segments. w2 of slot A is prefetched during phase 1; w2 of slot B
streams during phase 2's A blocks, so no phase transition stalls.
"""

import os
import sys

for _p in ("/opt/trn_rl_repo", "/root/.axon_site/_ro/trn_rl_repo"):
    if os.path.isdir(_p) and _p not in sys.path:
        sys.path.insert(0, _p)

import numpy as np

import concourse.bass as bass  # noqa: F401  (bass must import before bacc)
import concourse.mybir as mybir
import concourse.tile as tile
from concourse import bacc
from concourse.bass_utils import run_bass_kernel_spmd

H = 1024
E = 8
I = 4096
TOP_K = 2
P = 128
NTOK = 512  # token tile width in phase 1 (PSUM bank = 512 fp32)
F32 = mybir.dt.float32
AF = mybir.ActivationFunctionType

_programs: dict = {}


def _tile_widths(span):
    """Split a column span into phase-1 tile widths (512s + remainder)."""
    out = [NTOK] * (span // NTOK)
    if span % NTOK:
        out.append(span % NTOK)
    return out


def build_program(kA: int, kB: int, repeat: int = 1, dtype: str = "bf16",
                  ph: str = "both", out_reps: int | None = None) -> "bacc.Bacc":
    """One-core SPMD program: SwiGLU FFN for two token segments.

    Slot A = kA*128 token columns of expert A, slot B = kB*128 columns of
    expert B (kB may be 0: single-slot program, used by the fallback
    path). repeat > 1 re-emits the whole computation (benchmarking aid);
    out_reps pads the output allocation so benchmark variants share
    identical output shapes (the axon tunnel dispatch latency keys on
    transfer sizes).
    """
    DT = {"f32": F32, "f32r": mybir.dt.float32r, "bf16": mybir.dt.bfloat16}[dtype]
    spans = [kA * P] + ([kB * P] if kB else [])
    C = sum(spans)
    Cb = C // P
    HB = H // P  # 8
    IB = I // P  # 32
    # per-slot phase-1 token tiles: (slot, start, width)
    tts = []
    off = 0
    for s, span in enumerate(spans):
        for w in _tile_widths(span):
            tts.append((s, off, w))
            off += w
    nslots = len(spans)

    nc = bacc.Bacc("TRN2", target_bir_lowering=False, debug=False, num_devices=8)
    x_d = nc.dram_tensor("xt", [H, C], DT, kind="ExternalInput")
    w1_ds = [nc.dram_tensor(f"w1{'ab'[s]}", [P, IB, HB, P], DT, kind="ExternalInput")
             for s in range(nslots)]
    w3_ds = [nc.dram_tensor(f"w3{'ab'[s]}", [P, IB, HB, P], DT, kind="ExternalInput")
             for s in range(nslots)]
    w2_ds = [nc.dram_tensor(f"w2{'ab'[s]}", [I, H], DT, kind="ExternalInput")
             for s in range(nslots)]
    s_d = nc.dram_tensor("st", [P, Cb], F32, kind="ExternalInput")
    out_reps = max(out_reps or repeat, repeat)
    y_d = nc.dram_tensor("y", [out_reps * C, H], F32, kind="ExternalOutput")
    # h_act scratch, token-tile-major so phase-1 writes land as one wide
    # [P, <=512] store and phase-2 reads come in >=256B segments
    ha_d = nc.dram_tensor("hact", [len(tts), IB, P, NTOK], DT)

    xr = x_d.rearrange("(h p) c -> h p c", p=P)  # [8, 128, C]
    w2rs = [w2_d.rearrange("(i p) n -> p i n", p=P) for w2_d in w2_ds]  # [128,32,1024]

    # phase-2 token blocks: (block, slot, tile, col-within-tile)
    blocks = []
    for t, (s, st0, w) in enumerate(tts):
        for j in range(w // P):
            blocks.append((st0 // P + j, s, t, j * P))

    # phase-1 matmul groups: per slot, chunks of <=3 tiles (6 PSUM banks)
    slot_groups = []
    for s in range(nslots):
        stile = [t for t in range(len(tts)) if tts[t][0] == s]
        for g in range(0, len(stile), 3):
            slot_groups.append((s, stile[g: g + 3]))
    # i=0 runs tile 0 alone first: its x chunk lands earliest, so the PE
    # starts while the rest of x is still in flight
    slot_groups_i0 = slot_groups
    if len(slot_groups[0][1]) > 1:
        s0, g0 = slot_groups[0]
        slot_groups_i0 = [(s0, g0[:1]), (s0, g0[1:])] + slot_groups[1:]

    with tile.TileContext(nc) as tc:
      for rep in range(repeat):
        # pools are scoped to phases so phase-2's w2b stack reuses the SBUF
        # region freed by phase-1's x tiles (both resident would overflow)
        with (
            tc.tile_pool(name=f"ps1{rep}", bufs=8, space="PSUM") as ps_pool,
            tc.tile_pool(name=f"w2{rep}", bufs=1) as w2_pool,
            tc.tile_pool(name=f"sc{rep}", bufs=1) as s_pool,
            tc.tile_pool(name=f"hp{rep}", bufs=1) as hp_pool,
        ):
          with (
            tc.tile_pool(name=f"xt{rep}", bufs=1) as xt_pool,
            tc.tile_pool(name=f"w13{rep}", bufs=8) as w13_pool,
            tc.tile_pool(name=f"tmp{rep}", bufs=4) as tmp_pool,
            tc.tile_pool(name=f"hst{rep}", bufs=4) as hst_pool,
          ):
            # ---------------- phase 1: h = silu(x@w1) * (x@w3) ----------------
            w2ts = {s: [] for s in range(nslots)}
            hpins = {}
            if ph in ("both", "p1"):
                # issue order tracks the PE's critical path: slot-A w13[i=0],
                # then x in three column chunks (first A tile, rest of A, B)
                # so the first matmuls fire as early as possible
                w13ts = {}

                def load_w13(i, slots=None):
                    tl = w13ts.setdefault(i, {})
                    for s in (range(nslots) if slots is None else slots):
                        w1t = w13_pool.tile([P, HB, P], DT, tag="w13",
                                            name=f"w1_{'ab'[s]}{i}")
                        w3t = w13_pool.tile([P, HB, P], DT, tag="w13",
                                            name=f"w3_{'ab'[s]}{i}")
                        nc.sync.dma_start(out=w1t[:], in_=w1_ds[s][:, i])
                        nc.sync.dma_start(out=w3t[:], in_=w3_ds[s][:, i])
                        tl[s] = (w1t, w3t)

                load_w13(0, [0])
                W0 = min(NTOK, spans[0])  # first A tile
                W1 = spans[0]             # A/B boundary
                bnds = [0, W0, W1, C]
                xch = [[None] * 3 for _ in range(HB)]
                for ci in range(3):
                    lo, hi = bnds[ci], bnds[ci + 1]
                    if hi <= lo:
                        continue
                    for h in range(HB):
                        # alternate queues so early x tiles arrive twice as fast
                        eng = nc.scalar if h % 2 == 0 else nc.sync
                        xc = xt_pool.tile([P, hi - lo], DT, tag=f"x{h}_{ci}",
                                          name=f"x{h}_{ci}")
                        eng.dma_start(out=xc[:], in_=xr[h, :, lo:hi])
                        xch[h][ci] = xc
                    if ci == 0 and nslots > 1:
                        load_w13(0, [1])

                def xs(t, h):
                    """x slice [P, w] for token tile t, h-block h."""
                    _, st0, w = tts[t]
                    for ci in range(3):
                        if st0 < bnds[ci + 1]:
                            return xch[h][ci][:, st0 - bnds[ci]:
                                              st0 - bnds[ci] + w]
                    raise AssertionError

                load_w13(1)
                if ph == "both":
                    st = s_pool.tile([P, Cb], F32, tag="st", name="st")
                    nc.scalar.dma_start(out=st[:], in_=s_d[:])

                for i in range(IB):
                    if i not in w13ts:
                        load_w13(i)
                    # phase-2 slot-A weight prefetch, 2 tiles per iteration
                    if ph == "both" and i < IB // 2:
                        for j in (2 * i, 2 * i + 1):
                            w2t = w2_pool.tile([P, H], DT, tag=f"w2a_{j}",
                                               name=f"w2a_{j}")
                            nc.scalar.dma_start(out=w2t[:], in_=w2rs[0][:, j, :])
                            w2ts[0].append(w2t)
                    # per slot: w1 matmuls for all its tiles, then w3 — each
                    # stationary serves the whole slot span before switching.
                    # Slots wider than 3 tiles are chunked (PSUM = 8 banks).
                    for s, stile in (slot_groups_i0 if i == 0 else slot_groups):
                        w1t, w3t = w13ts[i][s]
                        p1s, p3s = {}, {}
                        for t in stile:
                            w = tts[t][2]
                            p1s[t] = ps_pool.tile([P, NTOK], F32, tag="ps",
                                                  name=f"p1_{i}_{t}")
                            p3s[t] = ps_pool.tile([P, NTOK], F32, tag="ps",
                                                  name=f"p3_{i}_{t}")
                        for h in range(HB):
                            for t in stile:
                                nc.tensor.matmul(
                                    p1s[t][:, : tts[t][2]], w1t[:, h, :], xs(t, h),
                                    start=(h == 0), stop=(h == HB - 1),
                                )
                        for h in range(HB):
                            for t in stile:
                                nc.tensor.matmul(
                                    p3s[t][:, : tts[t][2]], w3t[:, h, :], xs(t, h),
                                    start=(h == 0), stop=(h == HB - 1),
                                )
                        for t in stile:
                            w, p1, p3 = tts[t][2], p1s[t], p3s[t]
                            tmp = tmp_pool.tile([P, NTOK], F32, tag="tmp",
                                                name=f"tmp_{i}_{t}")
                            nc.scalar.activation(tmp[:, :w], p1[:, :w], AF.Silu)
                            if t == 0 and ph == "both":
                                # token tile 0's h stays pinned in SBUF: phase 2
                                # starts on it with no DMA dependency, hiding
                                # the hld loads for later tiles
                                hst = hp_pool.tile([P, NTOK], DT, tag=f"hp{i}",
                                                   name=f"hp_{i}")
                                hpins[i] = hst
                                nc.vector.tensor_mul(hst[:, :w], tmp[:, :w], p3[:, :w])
                            else:
                                hst = hst_pool.tile([P, NTOK], DT, tag="hst",
                                                    name=f"h_{i}_{t}")
                                nc.vector.tensor_mul(hst[:, :w], tmp[:, :w], p3[:, :w])
                                nc.sync.dma_start(out=ha_d[t, i, :, :w],
                                                  in_=hst[:, :w])

          # ---------------- phase 2: y = (h @ w2) * s ----------------
          with (
            tc.tile_pool(name=f"w2b{rep}", bufs=1) as w2b_pool,
            tc.tile_pool(name=f"hld{rep}", bufs=2) as hld_pool,
            tc.tile_pool(name=f"ysb{rep}", bufs=3) as y_pool,
          ):
            if ph in ("both", "p2"):
                if ph == "p2":
                    st = s_pool.tile([P, Cb], F32, tag="st", name="st")
                    nc.sync.dma_start(out=st[:], in_=s_d[:])
                    for i in range(IB):
                        w2t = w2_pool.tile([P, H], DT, tag=f"w2a_{i}",
                                           name=f"w2a_{i}")
                        nc.sync.dma_start(out=w2t[:], in_=w2rs[0][:, i, :])
                        w2ts[0].append(w2t)
                NH = H // NTOK  # 2
                # slot-B w2 streams in while the A blocks compute
                if nslots > 1:
                    for j in range(IB):
                        w2t = w2b_pool.tile([P, H], DT, tag=f"w2b_{j}",
                                            name=f"w2b_{j}")
                        nc.scalar.dma_start(out=w2t[:], in_=w2rs[1][:, j, :])
                        w2ts[1].append(w2t)
                # hld chunks: per tile, 2 token blocks (256 cols) per DMA
                cur_hld = {}
                for bi, (cb, s, t, cc) in enumerate(blocks):
                    pinned = t == 0 and hpins
                    if not pinned and (t, cc // (2 * P)) not in cur_hld:
                        w = tts[t][2]
                        j = cc // (2 * P)
                        cw = min(2 * P, w - j * 2 * P)
                        hld = hld_pool.tile([P, IB, 2 * P], DT, tag="hld",
                                            name=f"hld_{t}_{j}")
                        # sync queue: keeps hld clear of the w2b stream (scalar)
                        nc.sync.dma_start(
                            out=hld[:, :, :cw],
                            in_=ha_d[t, :, :, j * 2 * P: j * 2 * P + cw]
                            .rearrange("i p c -> p i c"))
                        cur_hld[(t, j)] = hld
                    yps = [
                        ps_pool.tile([P, NTOK], F32, tag="ps", name=f"yp_{cb}_{n}")
                        for n in range(NH)
                    ]
                    last = bi == len(blocks) - 1
                    k = (cc // P) % 2
                    if last:
                        # n-outer for the final block: n=0's epilogue overlaps
                        # n=1's matmuls, shrinking the tail
                        for n in range(NH):
                            for i in range(IB):
                                hsrc = (hpins[i][:, cc: cc + P] if pinned
                                        else cur_hld[(t, cc // (2 * P))]
                                        [:, i, k * P: (k + 1) * P])
                                nc.tensor.matmul(
                                    yps[n][:], hsrc,
                                    w2ts[s][i][:, n * NTOK: (n + 1) * NTOK],
                                    start=(i == 0), stop=(i == IB - 1),
                                )
                    else:
                        for i in range(IB):
                            hsrc = (hpins[i][:, cc: cc + P] if pinned
                                    else cur_hld[(t, cc // (2 * P))]
                                    [:, i, k * P: (k + 1) * P])
                            for n in range(NH):
                                nc.tensor.matmul(
                                    yps[n][:], hsrc,
                                    w2ts[s][i][:, n * NTOK: (n + 1) * NTOK],
                                    start=(i == 0), stop=(i == IB - 1),
                                )
                    for n in range(NH):
                        # final block: 256-wide chunks, scale-copy split across
                        # ACT and DVE, stores across both queues — drains the
                        # tail ~2x faster
                        nch = 2 if last else 1
                        for q in range(nch):
                            wq = NTOK // nch
                            ysb = y_pool.tile([P, wq], F32, tag="ysb",
                                              name=f"y_{cb}_{n}_{q}")
                            src = yps[n][:, q * wq: (q + 1) * wq]
                            if last and n == NH - 1 and q == 1:
                                nc.vector.tensor_scalar_mul(
                                    ysb[:], src, st[:, cb: cb + 1])
                                deng = nc.scalar
                            else:
                                nc.scalar.activation(
                                    ysb[:], src, AF.Copy,
                                    scale=st[:, cb: cb + 1])
                                deng = nc.sync
                            deng.dma_start(
                                out=y_d[
                                    rep * C + cb * P: rep * C + (cb + 1) * P,
                                    n * NTOK + q * wq: n * NTOK + (q + 1) * wq,
                                ],
                                in_=ysb[:],
                            )

    nc.compile()
    return nc


DTYPE = os.environ.get("MOE_DTYPE", "bf16")


def get_program(kA: int, kB: int) -> "bacc.Bacc":
    key = (kA, kB, DTYPE)
    if key not in _programs:
        _programs[key] = build_program(kA, kB, dtype=DTYPE)
    return _programs[key]


def _gate(x: np.ndarray, gate_w: np.ndarray):
    """Top-2 routing, mirroring the jax reference (softmax -> top_k ->
    renormalize). Uses jax for bit-compatible selection when available."""
    try:
        import jax
        import jax.numpy as jnp

        logits = jnp.asarray(x) @ jnp.asarray(gate_w)
        probs = jax.nn.softmax(logits, axis=-1)
        top_vals, top_idx = jax.lax.top_k(probs, TOP_K)
        top_vals = top_vals / jnp.sum(top_vals, axis=-1, keepdims=True)
        return np.asarray(top_vals), np.asarray(top_idx)
    except Exception:
        logits = x @ gate_w
        m = logits.max(-1, keepdims=True)
        p = np.exp(logits - m)
        p /= p.sum(-1, keepdims=True)
        top_idx = np.argsort(-p, axis=-1, kind="stable")[:, :TOP_K]
        top_vals = np.take_along_axis(p, top_idx, axis=-1)
        top_vals = top_vals / top_vals.sum(-1, keepdims=True)
        return top_vals, top_idx


def _route(x, gate_w):
    """Per-expert token index lists and routing weights."""
    top_vals, top_idx = _gate(x, gate_w)
    idxs, wts = [], []
    for e in range(E):
        sel = top_idx == e  # [T, K] bool
        mask = sel.any(axis=-1)
        idx_e = np.nonzero(mask)[0]
        w_e = np.where(sel[idx_e, 0], top_vals[idx_e, 0], top_vals[idx_e, 1])
        idxs.append(idx_e)
        wts.append(w_e.astype(np.float32))
    return idxs, wts


def _decompose(t_tiles, kA, kB):
    """Find per-expert (a_e, b_e) with sum(a)=sum(b)=8 and
    kA*a_e + kB*b_e >= t_e. Returns list of (a, b) or None."""
    nE = len(t_tiles)
    # DP over experts on (sum_a, sum_b)
    reach = {(0, 0): []}
    for e in range(nE):
        nxt = {}
        for (sa, sb), path in reach.items():
            for a in range(0, 9 - sa):
                # minimal b for this a
                need = t_tiles[e] - kA * a
                bmin = max(0, -(-need // kB))
                for b in range(bmin, 9 - sb):
                    k = (sa + a, sb + b)
                    if k not in nxt:
                        nxt[k] = path + [(a, b)]
        reach = nxt
        if not reach:
            return None
    return reach.get((8, 8))


def plan_dispatch(x, gate_w):
    """Balanced plan: per-core (expert_a, cols_a, expert_b, cols_b, scales).

    cols_* are padded token-id arrays (-1 = padding). Returns
    (kA, kB, per_core, idxs) or None if no balanced decomposition exists.
    """
    idxs, wts = _route(x, gate_w)
    t_tiles = [-(-max(len(ix), 1) // P) for ix in idxs]
    kT = -(-sum(t_tiles) // 8)
    found = None
    for kT_try in range(kT, kT + 3):
        for kA in range(kT_try - kT_try // 2, kT_try):
            kB = kT_try - kA
            ab = _decompose(t_tiles, kA, kB)
            if ab is not None:
                found = (kA, kB, ab)
                break
        if found:
            break
    if found is None:
        # single-slot fallback: one expert per core, capacity = max count
        kA, kB, ab = max(t_tiles), 0, [(1, 0)] * E
    else:
        kA, kB, ab = found

    # build pieces: expert e -> a_e chunks of kA*P slots, b_e of kB*P
    piecesA, piecesB = [], []
    for e in range(E):
        a, b = ab[e]
        padded = np.full(kA * P * a + kB * P * b, -1, np.int64)
        padded[: len(idxs[e])] = idxs[e]
        wpad = np.zeros(len(padded), np.float32)
        wpad[: len(idxs[e])] = wts[e]
        off = 0
        for _ in range(a):
            piecesA.append((e, padded[off: off + kA * P], wpad[off: off + kA * P]))
            off += kA * P
        for _ in range(b):
            piecesB.append((e, padded[off: off + kB * P], wpad[off: off + kB * P]))
            off += kB * P
    assert len(piecesA) == 8 and len(piecesB) in (0, 8)
    if kB == 0:
        empty = (np.empty(0, np.int64), np.zeros(0, np.float32))
        piecesB = [(piecesA[c][0],) + empty for c in range(8)]
    per_core = [piecesA[c] + piecesB[c] for c in range(8)]
    return kA, kB, per_core


def _block_w13(w):
    """[H, I] -> [P, IB, HB, P]: per-partition-contiguous stationary tiles."""
    return np.ascontiguousarray(
        w.reshape(H // P, P, I // P, P).transpose(1, 2, 0, 3))


_blocked_cache: dict = {}


def _npdt(dtype):
    if dtype == "bf16":
        import ml_dtypes
        return ml_dtypes.bfloat16
    return np.float32


def make_in_maps(x, w1, w3, w2, kA, kB, per_core, dtype=None):
    dtype = dtype or DTYPE
    npdt = _npdt(dtype)
    C = (kA + kB) * P
    in_maps = []
    w1b, w3b, w2b = {}, {}, {}

    def blocked(e):
        if e not in w1b:
            w1b[e] = _block_w13(np.asarray(w1[e], np.float32)).astype(npdt)
            w3b[e] = _block_w13(np.asarray(w3[e], np.float32)).astype(npdt)
            w2b[e] = np.ascontiguousarray(np.asarray(w2[e], np.float32)).astype(npdt)
        return w1b[e], w3b[e], w2b[e]

    for c in range(8):
        ea, cols_a, wts_a, eb, cols_b, wts_b = per_core[c]
        cols = np.concatenate([cols_a, cols_b])
        wcol = np.concatenate([wts_a, wts_b])
        x_pad = np.zeros((C, H), np.float32)
        real = cols >= 0
        x_pad[real] = x[cols[real]]
        w1a_, w3a_, w2a_ = blocked(ea)
        im = {
            "xt": np.ascontiguousarray(x_pad.T).astype(npdt),
            "w1a": w1a_, "w3a": w3a_, "w2a": w2a_,
            "st": np.ascontiguousarray(wcol.reshape(C // P, P).T),
        }
        if kB:
            im["w1b"], im["w3b"], im["w2b"] = blocked(eb)
        in_maps.append(im)
    return in_maps


def combine(results, per_core, T):
    out = np.zeros((T, H), np.float32)
    for c in range(8):
        ea, cols_a, _, eb, cols_b, _ = per_core[c]
        y = results[c]["y"]
        na = len(cols_a)
        for cols, rows in ((cols_a, y[:na]), (cols_b, y[na:])):
            real = cols >= 0
            out[cols[real]] += rows[real]
    return out


def kernel(hidden_states, gate_w, w1, w3, w2):
    B, S, Hh = hidden_states.shape
    assert Hh == H
    x = np.ascontiguousarray(hidden_states.reshape(-1, H), dtype=np.float32)
    T = x.shape[0]

    plan = plan_dispatch(x, gate_w)
    assert plan is not None, "no balanced decomposition; routing too skewed"
    kA, kB, per_core = plan
    nc = get_program(kA, kB)
    in_maps = make_in_maps(hidden_states.reshape(-1, H), w1, w3, w2,
                           kA, kB, per_core)
    res = run_bass_kernel_spmd(nc, in_maps, list(range(E)))
    out = combine(res.results, per_core, T)
    return out.reshape(B, S, H)


# revision 16
# speedup vs baseline: 1.0127x; 1.0127x over previous
"""MoE layer (top-2 routing, SwiGLU experts) for Trainium2, 8 NeuronCores.

Strategy: balanced expert parallelism. The gate (0.03% of FLOPs) and the
token dispatch/combine run on host. Tokens routed to each expert are
padded to 128-token tiles; the global tile count is split evenly across
the 8 cores (capacity = ceil(total_tiles/8) tiles/core instead of the
max-loaded expert), so every core does near-identical PE work. Each core
processes TWO token segments — slot A (kA tiles) and slot B (kB tiles)
with a GLOBAL static boundary — each segment belonging to one expert
whose weights are per-core inputs. Expert tile counts decompose into
kA/kB-sized pieces (e.g. 16=8+8, 17=9+8, 18=9+9), found by a small DP;
if no decomposition exists the kernel falls back to plain one-expert-
per-core with capacity = max expert count.

Each core runs the dense SwiGLU FFN in bf16 (PSUM accumulation is fp32;
tolerance is 2e-2, bf16 lands ~4e-3):

  phase 1:  h = silu(x @ w1) * (x @ w3)      (spilled to DRAM, token-tiled)
  phase 2:  y = (h @ w2) * route_weight      (tokens on partitions)

x is passed transposed ([H, CAP]) so phase 1 streams with tokens on the
free dimension and phase 2 uses h (token columns) as the matmul
stationary — no on-device transposes. w1/w3 are passed pre-blocked
[P, IB, HB, P] so every DMA moves >=1KB-contiguous per-partition
segments. w2 of slot A is prefetched during phase 1; w2 of slot B
streams during phase 2's A blocks, so no phase transition stalls.
"""

import os
import sys

for _p in ("/opt/trn_rl_repo", "/root/.axon_site/_ro/trn_rl_repo"):
    if os.path.isdir(_p) and _p not in sys.path:
        sys.path.insert(0, _p)

import numpy as np

import concourse.bass as bass  # noqa: F401  (bass must import before bacc)
import concourse.mybir as mybir
import concourse.tile as tile
from concourse import bacc
from concourse.bass_utils import run_bass_kernel_spmd

H = 1024
E = 8
I = 4096
TOP_K = 2
P = 128
NTOK = 512  # token tile width in phase 1 (PSUM bank = 512 fp32)
F32 = mybir.dt.float32
AF = mybir.ActivationFunctionType

_programs: dict = {}


def _tile_widths(span):
    """Split a column span into phase-1 tile widths (512s + remainder)."""
    out = [NTOK] * (span // NTOK)
    if span % NTOK:
        out.append(span % NTOK)
    return out


def build_program(kA: int, kB: int, repeat: int = 1, dtype: str = "bf16",
                  ph: str = "both", out_reps: int | None = None) -> "bacc.Bacc":
    """One-core SPMD program: SwiGLU FFN for two token segments.

    Slot A = kA*128 token columns of expert A, slot B = kB*128 columns of
    expert B (kB may be 0: single-slot program, used by the fallback
    path). repeat > 1 re-emits the whole computation (benchmarking aid);
    out_reps pads the output allocation so benchmark variants share
    identical output shapes (the axon tunnel dispatch latency keys on
    transfer sizes).
    """
    DT = {"f32": F32, "f32r": mybir.dt.float32r, "bf16": mybir.dt.bfloat16}[dtype]
    spans = [kA * P] + ([kB * P] if kB else [])
    C = sum(spans)
    Cb = C // P
    HB = H // P  # 8
    IB = I // P  # 32
    # per-slot phase-1 token tiles: (slot, start, width)
    tts = []
    off = 0
    for s, span in enumerate(spans):
        for w in _tile_widths(span):
            tts.append((s, off, w))
            off += w
    nslots = len(spans)

    nc = bacc.Bacc("TRN2", target_bir_lowering=False, debug=False, num_devices=8)
    x_d = nc.dram_tensor("xt", [H, C], DT, kind="ExternalInput")
    w1_ds = [nc.dram_tensor(f"w1{'ab'[s]}", [P, IB, HB, P], DT, kind="ExternalInput")
             for s in range(nslots)]
    w3_ds = [nc.dram_tensor(f"w3{'ab'[s]}", [P, IB, HB, P], DT, kind="ExternalInput")
             for s in range(nslots)]
    w2_ds = [nc.dram_tensor(f"w2{'ab'[s]}", [I, H], DT, kind="ExternalInput")
             for s in range(nslots)]
    s_d = nc.dram_tensor("st", [P, Cb], F32, kind="ExternalInput")
    out_reps = max(out_reps or repeat, repeat)
    y_d = nc.dram_tensor("y", [out_reps * C, H], F32, kind="ExternalOutput")
    # h_act scratch, token-tile-major so phase-1 writes land as one wide
    # [P, <=512] store and phase-2 reads come in >=256B segments
    ha_d = nc.dram_tensor("hact", [len(tts), IB, P, NTOK], DT)

    xr = x_d.rearrange("(h p) c -> h p c", p=P)  # [8, 128, C]
    w2rs = [w2_d.rearrange("(i p) n -> p i n", p=P) for w2_d in w2_ds]  # [128,32,1024]

    # phase-2 token blocks: (block, slot, tile, col-within-tile)
    blocks = []
    for t, (s, st0, w) in enumerate(tts):
        for j in range(w // P):
            blocks.append((st0 // P + j, s, t, j * P))

    # phase-1 matmul groups: per slot, chunks of <=3 tiles (6 PSUM banks)
    slot_groups = []
    for s in range(nslots):
        stile = [t for t in range(len(tts)) if tts[t][0] == s]
        for g in range(0, len(stile), 3):
            slot_groups.append((s, stile[g: g + 3]))
    # i=0 runs tile 0 alone first: its x chunk lands earliest, so the PE
    # starts while the rest of x is still in flight
    slot_groups_i0 = slot_groups
    if len(slot_groups[0][1]) > 1:
        s0, g0 = slot_groups[0]
        slot_groups_i0 = [(s0, g0[:1]), (s0, g0[1:])] + slot_groups[1:]

    with tile.TileContext(nc) as tc:
      for rep in range(repeat):
        # pools are scoped to phases so phase-2's w2b stack reuses the SBUF
        # region freed by phase-1's x tiles (both resident would overflow)
        with (
            tc.tile_pool(name=f"ps1{rep}", bufs=8, space="PSUM") as ps_pool,
            tc.tile_pool(name=f"w2{rep}", bufs=1) as w2_pool,
            tc.tile_pool(name=f"sc{rep}", bufs=1) as s_pool,
            tc.tile_pool(name=f"hp{rep}", bufs=1) as hp_pool,
        ):
          with (
            tc.tile_pool(name=f"xt{rep}", bufs=1) as xt_pool,
            tc.tile_pool(name=f"w13{rep}", bufs=8) as w13_pool,
            tc.tile_pool(name=f"tmp{rep}", bufs=4) as tmp_pool,
            tc.tile_pool(name=f"hst{rep}", bufs=4) as hst_pool,
          ):
            # ---------------- phase 1: h = silu(x@w1) * (x@w3) ----------------
            w2ts = {s: [] for s in range(nslots)}
            hpins = {}
            if ph in ("both", "p1"):
                # issue order tracks the PE's critical path: slot-A w13[i=0],
                # then x in three column chunks (first A tile, rest of A, B)
                # so the first matmuls fire as early as possible
                w13ts = {}

                def load_w13(i, slots=None):
                    tl = w13ts.setdefault(i, {})
                    for s in (range(nslots) if slots is None else slots):
                        w1t = w13_pool.tile([P, HB, P], DT, tag="w13",
                                            name=f"w1_{'ab'[s]}{i}")
                        w3t = w13_pool.tile([P, HB, P], DT, tag="w13",
                                            name=f"w3_{'ab'[s]}{i}")
                        nc.sync.dma_start(out=w1t[:], in_=w1_ds[s][:, i])
                        nc.sync.dma_start(out=w3t[:], in_=w3_ds[s][:, i])
                        tl[s] = (w1t, w3t)

                load_w13(0, [0])
                W0 = min(NTOK, spans[0])  # first A tile
                W1 = spans[0]             # A/B boundary
                bnds = [0, W0, W1, C]
                xch = [[None] * 3 for _ in range(HB)]
                for ci in range(3):
                    lo, hi = bnds[ci], bnds[ci + 1]
                    if hi <= lo:
                        continue
                    for h in range(HB):
                        # alternate queues so early x tiles arrive twice as fast
                        eng = nc.scalar if h % 2 == 0 else nc.sync
                        xc = xt_pool.tile([P, hi - lo], DT, tag=f"x{h}_{ci}",
                                          name=f"x{h}_{ci}")
                        eng.dma_start(out=xc[:], in_=xr[h, :, lo:hi])
                        xch[h][ci] = xc
                    if ci == 0 and nslots > 1:
                        load_w13(0, [1])

                def xs(t, h):
                    """x slice [P, w] for token tile t, h-block h."""
                    _, st0, w = tts[t]
                    for ci in range(3):
                        if st0 < bnds[ci + 1]:
                            return xch[h][ci][:, st0 - bnds[ci]:
                                              st0 - bnds[ci] + w]
                    raise AssertionError

                load_w13(1)
                if ph == "both":
                    st = s_pool.tile([P, Cb], F32, tag="st", name="st")
                    nc.scalar.dma_start(out=st[:], in_=s_d[:])

                for i in range(IB):
                    if i not in w13ts:
                        load_w13(i)
                    # phase-2 slot-A weight prefetch, 2 tiles per iteration
                    if ph == "both" and i < IB // 2:
                        for j in (2 * i, 2 * i + 1):
                            w2t = w2_pool.tile([P, H], DT, tag=f"w2a_{j}",
                                               name=f"w2a_{j}")
                            nc.scalar.dma_start(out=w2t[:], in_=w2rs[0][:, j, :])
                            w2ts[0].append(w2t)
                    # per slot: w1 matmuls for all its tiles, then w3 — each
                    # stationary serves the whole slot span before switching.
                    # Slots wider than 3 tiles are chunked (PSUM = 8 banks).
                    for s, stile in (slot_groups_i0 if i == 0 else slot_groups):
                        w1t, w3t = w13ts[i][s]
                        p1s, p3s = {}, {}
                        for t in stile:
                            w = tts[t][2]
                            p1s[t] = ps_pool.tile([P, NTOK], F32, tag="ps",
                                                  name=f"p1_{i}_{t}")
                            p3s[t] = ps_pool.tile([P, NTOK], F32, tag="ps",
                                                  name=f"p3_{i}_{t}")
                        for h in range(HB):
                            for t in stile:
                                nc.tensor.matmul(
                                    p1s[t][:, : tts[t][2]], w1t[:, h, :], xs(t, h),
                                    start=(h == 0), stop=(h == HB - 1),
                                )
                        for h in range(HB):
                            for t in stile:
                                nc.tensor.matmul(
                                    p3s[t][:, : tts[t][2]], w3t[:, h, :], xs(t, h),
                                    start=(h == 0), stop=(h == HB - 1),
                                )
                        for t in stile:
                            w, p1, p3 = tts[t][2], p1s[t], p3s[t]
                            tmp = tmp_pool.tile([P, NTOK], F32, tag="tmp",
                                                name=f"tmp_{i}_{t}")
                            nc.scalar.activation(tmp[:, :w], p1[:, :w], AF.Silu)
                            if t == 0 and ph == "both":
                                # token tile 0's h stays pinned in SBUF: phase 2
                                # starts on it with no DMA dependency, hiding
                                # the hld loads for later tiles
                                hst = hp_pool.tile([P, NTOK], DT, tag=f"hp{i}",
                                                   name=f"hp_{i}")
                                hpins[i] = hst
                                nc.vector.tensor_mul(hst[:, :w], tmp[:, :w], p3[:, :w])
                            else:
                                hst = hst_pool.tile([P, NTOK], DT, tag="hst",
                                                    name=f"h_{i}_{t}")
                                nc.vector.tensor_mul(hst[:, :w], tmp[:, :w], p3[:, :w])
                                nc.sync.dma_start(out=ha_d[t, i, :, :w],
                                                  in_=hst[:, :w])

          # ---------------- phase 2: y = (h @ w2) * s ----------------
          with (
            tc.tile_pool(name=f"w2b{rep}", bufs=1) as w2b_pool,
            tc.tile_pool(name=f"hld{rep}", bufs=2) as hld_pool,
            tc.tile_pool(name=f"ysb{rep}", bufs=3) as y_pool,
          ):
            if ph in ("both", "p2"):
                if ph == "p2":
                    st = s_pool.tile([P, Cb], F32, tag="st", name="st")
                    nc.sync.dma_start(out=st[:], in_=s_d[:])
                    for i in range(IB):
                        w2t = w2_pool.tile([P, H], DT, tag=f"w2a_{i}",
                                           name=f"w2a_{i}")
                        nc.sync.dma_start(out=w2t[:], in_=w2rs[0][:, i, :])
                        w2ts[0].append(w2t)
                NH = H // NTOK  # 2
                # slot-B w2 streams in while the A blocks compute
                if nslots > 1:
                    for j in range(IB):
                        w2t = w2b_pool.tile([P, H], DT, tag=f"w2b_{j}",
                                            name=f"w2b_{j}")
                        nc.scalar.dma_start(out=w2t[:], in_=w2rs[1][:, j, :])
                        w2ts[1].append(w2t)
                # hld chunks: per tile, 2 token blocks (256 cols) per DMA
                cur_hld = {}
                for bi, (cb, s, t, cc) in enumerate(blocks):
                    pinned = t == 0 and hpins
                    if not pinned and (t, cc // (2 * P)) not in cur_hld:
                        w = tts[t][2]
                        j = cc // (2 * P)
                        cw = min(2 * P, w - j * 2 * P)
                        hld = hld_pool.tile([P, IB, 2 * P], DT, tag="hld",
                                            name=f"hld_{t}_{j}")
                        # sync queue: keeps hld clear of the w2b stream (scalar)
                        nc.sync.dma_start(
                            out=hld[:, :, :cw],
                            in_=ha_d[t, :, :, j * 2 * P: j * 2 * P + cw]
                            .rearrange("i p c -> p i c"))
                        cur_hld[(t, j)] = hld
                    yps = [
                        ps_pool.tile([P, NTOK], F32, tag="ps", name=f"yp_{cb}_{n}")
                        for n in range(NH)
                    ]
                    last = bi == len(blocks) - 1
                    k = (cc // P) % 2
                    if last:
                        # n-outer for the final block: n=0's epilogue overlaps
                        # n=1's matmuls, shrinking the tail
                        for n in range(NH):
                            for i in range(IB):
                                hsrc = (hpins[i][:, cc: cc + P] if pinned
                                        else cur_hld[(t, cc // (2 * P))]
                                        [:, i, k * P: (k + 1) * P])
                                nc.tensor.matmul(
                                    yps[n][:], hsrc,
                                    w2ts[s][i][:, n * NTOK: (n + 1) * NTOK],
                                    start=(i == 0), stop=(i == IB - 1),
                                )
                    else:
                        for i in range(IB):
                            hsrc = (hpins[i][:, cc: cc + P] if pinned
                                    else cur_hld[(t, cc // (2 * P))]
                                    [:, i, k * P: (k + 1) * P])
                            for n in range(NH):
                                nc.tensor.matmul(
                                    yps[n][:], hsrc,
                                    w2ts[s][i][:, n * NTOK: (n + 1) * NTOK],
                                    start=(i == 0), stop=(i == IB - 1),
                                )
                    for n in range(NH):
                        # final block: 256-wide chunks, scale-copy split across
                        # ACT and DVE, stores across both queues — drains the
                        # tail ~2x faster
                        nch = 2 if last else 1
                        for q in range(nch):
                            wq = NTOK // nch
                            ysb = y_pool.tile([P, wq], F32, tag="ysb",
                                              name=f"y_{cb}_{n}_{q}")
                            src = yps[n][:, q * wq: (q + 1) * wq]
                            if last and n == NH - 1 and q == 1:
                                nc.vector.tensor_scalar_mul(
                                    ysb[:], src, st[:, cb: cb + 1])
                                deng = nc.scalar
                            else:
                                nc.scalar.activation(
                                    ysb[:], src, AF.Copy,
                                    scale=st[:, cb: cb + 1])
                                deng = nc.sync
                            deng.dma_start(
                                out=y_d[
                                    rep * C + cb * P: rep * C + (cb + 1) * P,
                                    n * NTOK + q * wq: n * NTOK + (q + 1) * wq,
                                ],
                                in_=ysb[:],
                            )

    nc.compile()
    return nc


DTYPE = os.environ.get("MOE_DTYPE", "bf16")


def get_program(kA: int, kB: int) -> "bacc.Bacc":
    key = (kA, kB, DTYPE)
    if key not in _programs:
        _programs[key] = build_program(kA, kB, dtype=DTYPE)
    return _programs[key]


def _gate(x: np.ndarray, gate_w: np.ndarray):
    """Top-2 routing, mirroring the jax reference (softmax -> top_k ->
    renormalize). Uses jax for bit-compatible selection when available."""
    try:
        import jax
        import jax.numpy as jnp

        logits = jnp.asarray(x) @ jnp.asarray(gate_w)
        probs = jax.nn.softmax(logits, axis=-1)
        top_vals, top_idx = jax.lax.top_k(probs, TOP_K)
        top_vals = top_vals / jnp.sum(top_vals, axis=-1, keepdims=True)
        return np.asarray(top_vals), np.asarray(top_idx)
    except Exception:
        logits = x @ gate_w
        m = logits.max(-1, keepdims=True)
        p = np.exp(logits - m)
        p /= p.sum(-1, keepdims=True)
        top_idx = np.argsort(-p, axis=-1, kind="stable")[:, :TOP_K]
        top_vals = np.take_along_axis(p, top_idx, axis=-1)
        top_vals = top_vals / top_vals.sum(-1, keepdims=True)
        return top_vals, top_idx


def _route(x, gate_w):
    """Per-expert token index lists and routing weights."""
    top_vals, top_idx = _gate(x, gate_w)
    idxs, wts = [], []
    for e in range(E):
        sel = top_idx == e  # [T, K] bool
        mask = sel.any(axis=-1)
        idx_e = np.nonzero(mask)[0]
        w_e = np.where(sel[idx_e, 0], top_vals[idx_e, 0], top_vals[idx_e, 1])
        idxs.append(idx_e)
        wts.append(w_e.astype(np.float32))
    return idxs, wts


def _decompose(t_tiles, kA, kB):
    """Find per-expert (a_e, b_e) with sum(a)=sum(b)=8 and
    kA*a_e + kB*b_e >= t_e. Returns list of (a, b) or None."""
    nE = len(t_tiles)
    # DP over experts on (sum_a, sum_b)
    reach = {(0, 0): []}
    for e in range(nE):
        nxt = {}
        for (sa, sb), path in reach.items():
            for a in range(0, 9 - sa):
                # minimal b for this a
                need = t_tiles[e] - kA * a
                bmin = max(0, -(-need // kB))
                for b in range(bmin, 9 - sb):
                    k = (sa + a, sb + b)
                    if k not in nxt:
                        nxt[k] = path + [(a, b)]
        reach = nxt
        if not reach:
            return None
    return reach.get((8, 8))


def plan_dispatch(x, gate_w):
    """Balanced plan: per-core (expert_a, cols_a, expert_b, cols_b, scales).

    cols_* are padded token-id arrays (-1 = padding). Returns
    (kA, kB, per_core, idxs) or None if no balanced decomposition exists.
    """
    idxs, wts = _route(x, gate_w)
    t_tiles = [-(-max(len(ix), 1) // P) for ix in idxs]
    kT = -(-sum(t_tiles) // 8)
    found = None
    for kT_try in range(kT, kT + 3):
        for kA in range(kT_try - kT_try // 2, kT_try):
            kB = kT_try - kA
            ab = _decompose(t_tiles, kA, kB)
            if ab is not None:
                found = (kA, kB, ab)
                break
        if found:
            break
    if found is None:
        # single-slot fallback: one expert per core, capacity = max count
        kA, kB, ab = max(t_tiles), 0, [(1, 0)] * E
    else:
        kA, kB, ab = found

    # build pieces: expert e -> a_e chunks of kA*P slots, b_e of kB*P
    piecesA, piecesB = [], []
    for e in range(E):
        a, b = ab[e]
        padded = np.full(kA * P * a + kB * P * b, -1, np.int64)
        padded[: len(idxs[e])] = idxs[e]
        wpad = np.zeros(len(padded), np.float32)
        wpad[: len(idxs[e])] = wts[e]
        off = 0
        for _ in range(a):
            piecesA.append((e, padded[off: off + kA * P], wpad[off: off + kA * P]))
            off += kA * P
        for _ in range(b):
            piecesB.append((e, padded[off: off + kB * P], wpad[off: off + kB * P]))
            off += kB * P
    assert len(piecesA) == 8 and len(piecesB) in (0, 8)
    if kB == 0:
        empty = (np.empty(0, np.int64), np.zeros(0, np.float32))
        piecesB = [(piecesA[c][0],) + empty for c in range(8)]
    per_core = [piecesA[c] + piecesB[c] for c in range(8)]
    return kA, kB, per_core


def _block_w13(w):
    """[H, I] -> [P, IB, HB, P]: per-partition-contiguous stationary tiles."""
    return np.ascontiguousarray(
        w.reshape(H // P, P, I // P, P).transpose(1, 2, 0, 3))


def _npdt(dtype):
    if dtype == "bf16":
        import ml_dtypes
        return ml_dtypes.bfloat16
    return np.float32


def make_in_maps(x, w1, w3, w2, kA, kB, per_core, dtype=None):
    dtype = dtype or DTYPE
    npdt = _npdt(dtype)
    C = (kA + kB) * P
    in_maps = []
    w1b, w3b, w2b = {}, {}, {}

    def blocked(e):
        if e not in w1b:
            w1b[e] = _block_w13(np.asarray(w1[e], np.float32)).astype(npdt)
            w3b[e] = _block_w13(np.asarray(w3[e], np.float32)).astype(npdt)
            w2b[e] = np.ascontiguousarray(np.asarray(w2[e], np.float32)).astype(npdt)
        return w1b[e], w3b[e], w2b[e]

    for c in range(8):
        ea, cols_a, wts_a, eb, cols_b, wts_b = per_core[c]
        cols = np.concatenate([cols_a, cols_b])
        wcol = np.concatenate([wts_a, wts_b])
        x_pad = np.zeros((C, H), np.float32)
        real = cols >= 0
        x_pad[real] = x[cols[real]]
        w1a_, w3a_, w2a_ = blocked(ea)
        im = {
            "xt": np.ascontiguousarray(x_pad.T).astype(npdt),
            "w1a": w1a_, "w3a": w3a_, "w2a": w2a_,
            "st": np.ascontiguousarray(wcol.reshape(C // P, P).T),
        }
        if kB:
            im["w1b"], im["w3b"], im["w2b"] = blocked(eb)
        in_maps.append(im)
    return in_maps


def combine(results, per_core, T):
    out = np.zeros((T, H), np.float32)
    for c in range(8):
        ea, cols_a, _, eb, cols_b, _ = per_core[c]
        y = results[c]["y"]
        na = len(cols_a)
        for cols, rows in ((cols_a, y[:na]), (cols_b, y[na:])):
            real = cols >= 0
            out[cols[real]] += rows[real]
    return out


def kernel(hidden_states, gate_w, w1, w3, w2):
    B, S, Hh = hidden_states.shape
    assert Hh == H
    x = np.ascontiguousarray(hidden_states.reshape(-1, H), dtype=np.float32)
    T = x.shape[0]

    plan = plan_dispatch(x, gate_w)
    assert plan is not None, "no balanced decomposition; routing too skewed"
    kA, kB, per_core = plan
    nc = get_program(kA, kB)
    in_maps = make_in_maps(hidden_states.reshape(-1, H), w1, w3, w2,
                           kA, kB, per_core)
    res = run_bass_kernel_spmd(nc, in_maps, list(range(E)))
    out = combine(res.results, per_core, T)
    return out.reshape(B, S, H)


# revision 23
# speedup vs baseline: 1.0425x; 1.0294x over previous
"""MoE layer (top-2 routing, SwiGLU experts) for Trainium2, 8 NeuronCores.

Strategy: balanced expert parallelism. The gate (0.03% of FLOPs) and the
token dispatch/combine run on host. Tokens routed to each expert are
padded to 128-token tiles; the global tile count is split evenly across
the 8 cores (capacity = ceil(total_tiles/8) tiles/core instead of the
max-loaded expert), so every core does near-identical PE work. Each core
processes TWO token segments — slot A (kA tiles) and slot B (kB tiles)
with a GLOBAL static boundary — each segment belonging to one expert
whose weights are per-core inputs. Expert tile counts decompose into
kA/kB-sized pieces (e.g. 16=8+8, 17=9+8, 18=9+9), found by a small DP;
if no decomposition exists the kernel falls back to plain one-expert-
per-core with capacity = max expert count.

Each core runs the dense SwiGLU FFN in bf16 (PSUM accumulation is fp32;
tolerance is 2e-2, bf16 lands ~4e-3):

  phase 1:  h = silu(x @ w1) * (x @ w3)      (spilled to DRAM, token-tiled)
  phase 2:  y = (h @ w2) * route_weight      (tokens on partitions)

x is passed transposed ([H, CAP]) so phase 1 streams with tokens on the
free dimension and phase 2 uses h (token columns) as the matmul
stationary — no on-device transposes. w1/w3 are passed pre-blocked
[P, IB, HB, P] so every DMA moves >=1KB-contiguous per-partition
segments. w2 of slot A is prefetched during phase 1; w2 of slot B
streams during phase 2's A blocks, so no phase transition stalls.
"""

import os
import sys

for _p in ("/opt/trn_rl_repo", "/root/.axon_site/_ro/trn_rl_repo"):
    if os.path.isdir(_p) and _p not in sys.path:
        sys.path.insert(0, _p)

import numpy as np

import concourse.bass as bass  # noqa: F401  (bass must import before bacc)
import concourse.mybir as mybir
import concourse.tile as tile
from concourse import bacc
from concourse.bass_utils import run_bass_kernel_spmd

H = 1024
E = 8
I = 4096
TOP_K = 2
P = 128
NTOK = 512  # token tile width in phase 1 (PSUM bank = 512 fp32)
F32 = mybir.dt.float32
AF = mybir.ActivationFunctionType

_programs: dict = {}


def _tile_widths(span):
    """Split a column span into phase-1 tile widths (512s + remainder)."""
    out = [NTOK] * (span // NTOK)
    if span % NTOK:
        out.append(span % NTOK)
    return out


def build_program(kA: int, kB: int, repeat: int = 1, dtype: str = "bf16",
                  ph: str = "both", out_reps: int | None = None) -> "bacc.Bacc":
    """One-core SPMD program: SwiGLU FFN for two token segments.

    Slot A = kA*128 token columns of expert A, slot B = kB*128 columns of
    expert B (kB may be 0: single-slot program, used by the fallback
    path). repeat > 1 re-emits the whole computation (benchmarking aid);
    out_reps pads the output allocation so benchmark variants share
    identical output shapes (the axon tunnel dispatch latency keys on
    transfer sizes).
    """
    DT = {"f32": F32, "f32r": mybir.dt.float32r, "bf16": mybir.dt.bfloat16}[dtype]
    spans = [kA * P] + ([kB * P] if kB else [])
    C = sum(spans)
    Cb = C // P
    HB = H // P  # 8
    IB = I // P  # 32
    # per-slot phase-1 token tiles: (slot, start, width)
    tts = []
    off = 0
    for s, span in enumerate(spans):
        for w in _tile_widths(span):
            tts.append((s, off, w))
            off += w
    nslots = len(spans)

    nc = bacc.Bacc("TRN2", target_bir_lowering=False, debug=False, num_devices=8)
    x_d = nc.dram_tensor("xt", [H, C], DT, kind="ExternalInput")
    w1_ds = [nc.dram_tensor(f"w1{'ab'[s]}", [P, IB, HB, P], DT, kind="ExternalInput")
             for s in range(nslots)]
    w3_ds = [nc.dram_tensor(f"w3{'ab'[s]}", [P, IB, HB, P], DT, kind="ExternalInput")
             for s in range(nslots)]
    w2_ds = [nc.dram_tensor(f"w2{'ab'[s]}", [I, H], DT, kind="ExternalInput")
             for s in range(nslots)]
    s_d = nc.dram_tensor("st", [P, Cb], F32, kind="ExternalInput")
    out_reps = max(out_reps or repeat, repeat)
    y_d = nc.dram_tensor("y", [out_reps * C, H], F32, kind="ExternalOutput")
    # h_act scratch, token-tile-major so phase-1 writes land as one wide
    # [P, <=512] store and phase-2 reads come in >=256B segments
    ha_d = nc.dram_tensor("hact", [len(tts), IB, P, NTOK], DT)

    xr = x_d.rearrange("(h p) c -> h p c", p=P)  # [8, 128, C]
    w2rs = [w2_d.rearrange("(i p) n -> p i n", p=P) for w2_d in w2_ds]  # [128,32,1024]

    # phase-2 token blocks: (block, slot, tile, col-within-tile)
    blocks = []
    for t, (s, st0, w) in enumerate(tts):
        for j in range(w // P):
            blocks.append((st0 // P + j, s, t, j * P))

    # phase-1 matmul groups: per slot, chunks of <=3 tiles (6 PSUM banks)
    slot_groups = []
    for s in range(nslots):
        stile = [t for t in range(len(tts)) if tts[t][0] == s]
        for g in range(0, len(stile), 3):
            slot_groups.append((s, stile[g: g + 3]))
    # i=0 runs tile 0 alone first: its x chunk lands earliest, so the PE
    # starts while the rest of x is still in flight
    slot_groups_i0 = slot_groups
    if len(slot_groups[0][1]) > 1:
        s0, g0 = slot_groups[0]
        slot_groups_i0 = [(s0, g0[:1]), (s0, g0[1:])] + slot_groups[1:]

    with tile.TileContext(nc) as tc:
     # program-lifetime pool for each rep's first x chunk + slot-A w13[0]:
     # their loads overlap the previous rep's phase 2 (the per-rep pools'
     # regions only free at rep end, which would serialize these loads)
     with tc.tile_pool(name="warm", bufs=1) as warm_pool:
      for rep in range(repeat):
        # pools are scoped to phases so phase-2's w2b stack reuses the SBUF
        # region freed by phase-1's x tiles (both resident would overflow)
        with (
            tc.tile_pool(name=f"ps1{rep}", bufs=8, space="PSUM") as ps_pool,
            tc.tile_pool(name=f"w2{rep}", bufs=1) as w2_pool,
            tc.tile_pool(name=f"sc{rep}", bufs=1) as s_pool,
            tc.tile_pool(name=f"hp{rep}", bufs=1) as hp_pool,
        ):
          with (
            tc.tile_pool(name=f"xt{rep}", bufs=1) as xt_pool,
            tc.tile_pool(name=f"w13{rep}", bufs=8) as w13_pool,
            tc.tile_pool(name=f"tmp{rep}", bufs=4) as tmp_pool,
            tc.tile_pool(name=f"hst{rep}", bufs=4) as hst_pool,
          ):
            # ---------------- phase 1: h = silu(x@w1) * (x@w3) ----------------
            w2ts = {s: [] for s in range(nslots)}
            hpins = {}
            if ph in ("both", "p1"):
                # issue order tracks the PE's critical path: slot-A w13[i=0],
                # then x in three column chunks (first A tile, rest of A, B)
                # so the first matmuls fire as early as possible
                w13ts = {}

                def load_w13(i, slots=None):
                    tl = w13ts.setdefault(i, {})
                    for s in (range(nslots) if slots is None else slots):
                        w1t = w13_pool.tile([P, HB, P], DT, tag="w13",
                                            name=f"w1_{'ab'[s]}{i}_{rep}")
                        w3t = w13_pool.tile([P, HB, P], DT, tag="w13",
                                            name=f"w3_{'ab'[s]}{i}_{rep}")
                        nc.sync.dma_start(out=w1t[:], in_=w1_ds[s][:, i])
                        nc.sync.dma_start(out=w3t[:], in_=w3_ds[s][:, i])
                        tl[s] = (w1t, w3t)

                load_w13(0, [0])
                W0 = min(NTOK, spans[0])  # first A tile
                W1 = spans[0]             # A/B boundary
                bnds = [0, W0, W1, C]
                xch = [[None] * 3 for _ in range(HB)]
                for ci in range(3):
                    lo, hi = bnds[ci], bnds[ci + 1]
                    if hi <= lo:
                        continue
                    for h in range(HB):
                        # alternate queues so early x tiles arrive twice as fast
                        eng = nc.scalar if h % 2 == 0 else nc.sync
                        pool = warm_pool if ci == 0 else xt_pool
                        xc = pool.tile([P, hi - lo], DT, tag=f"x{h}_{ci}",
                                       name=f"x{h}_{ci}_{rep}")
                        eng.dma_start(out=xc[:], in_=xr[h, :, lo:hi])
                        xch[h][ci] = xc
                    if ci == 0 and nslots > 1:
                        load_w13(0, [1])

                def xs(t, h):
                    """x slice [P, w] for token tile t, h-block h."""
                    _, st0, w = tts[t]
                    for ci in range(3):
                        if st0 < bnds[ci + 1]:
                            return xch[h][ci][:, st0 - bnds[ci]:
                                              st0 - bnds[ci] + w]
                    raise AssertionError

                load_w13(1)
                if ph == "both":
                    st = s_pool.tile([P, Cb], F32, tag="st", name="st")
                    nc.scalar.dma_start(out=st[:], in_=s_d[:])

                for i in range(IB):
                    if i not in w13ts:
                        load_w13(i)
                    # phase-2 slot-A weight prefetch, 2 tiles per iteration
                    if ph == "both" and i < IB // 2:
                        for j in (2 * i, 2 * i + 1):
                            w2t = w2_pool.tile([P, H], DT, tag=f"w2a_{j}",
                                               name=f"w2a_{j}")
                            nc.scalar.dma_start(out=w2t[:], in_=w2rs[0][:, j, :])
                            w2ts[0].append(w2t)
                    # per slot: w1 matmuls for all its tiles, then w3 — each
                    # stationary serves the whole slot span before switching.
                    # Slots wider than 3 tiles are chunked (PSUM = 8 banks).
                    for s, stile in (slot_groups_i0 if i == 0 else slot_groups):
                        w1t, w3t = w13ts[i][s]
                        p1s, p3s = {}, {}
                        for t in stile:
                            w = tts[t][2]
                            p1s[t] = ps_pool.tile([P, NTOK], F32, tag="ps",
                                                  name=f"p1_{i}_{t}")
                            p3s[t] = ps_pool.tile([P, NTOK], F32, tag="ps",
                                                  name=f"p3_{i}_{t}")
                        for h in range(HB):
                            for t in stile:
                                nc.tensor.matmul(
                                    p1s[t][:, : tts[t][2]], w1t[:, h, :], xs(t, h),
                                    start=(h == 0), stop=(h == HB - 1),
                                )
                        for h in range(HB):
                            for t in stile:
                                nc.tensor.matmul(
                                    p3s[t][:, : tts[t][2]], w3t[:, h, :], xs(t, h),
                                    start=(h == 0), stop=(h == HB - 1),
                                )
                        for t in stile:
                            w, p1, p3 = tts[t][2], p1s[t], p3s[t]
                            tmp = tmp_pool.tile([P, NTOK], F32, tag="tmp",
                                                name=f"tmp_{i}_{t}")
                            nc.scalar.activation(tmp[:, :w], p1[:, :w], AF.Silu)
                            if t == 0 and ph == "both":
                                # token tile 0's h stays pinned in SBUF: phase 2
                                # starts on it with no DMA dependency, hiding
                                # the hld loads for later tiles
                                hst = hp_pool.tile([P, NTOK], DT, tag=f"hp{i}",
                                                   name=f"hp_{i}")
                                hpins[i] = hst
                                nc.vector.tensor_mul(hst[:, :w], tmp[:, :w], p3[:, :w])
                            else:
                                hst = hst_pool.tile([P, NTOK], DT, tag="hst",
                                                    name=f"h_{i}_{t}")
                                nc.vector.tensor_mul(hst[:, :w], tmp[:, :w], p3[:, :w])
                                nc.sync.dma_start(out=ha_d[t, i, :, :w],
                                                  in_=hst[:, :w])

          # ---------------- phase 2: y = (h @ w2) * s ----------------
          with (
            tc.tile_pool(name=f"w2b{rep}", bufs=1) as w2b_pool,
            tc.tile_pool(name=f"hld{rep}", bufs=2) as hld_pool,
            tc.tile_pool(name=f"ysb{rep}", bufs=3) as y_pool,
          ):
            if ph in ("both", "p2"):
                if ph == "p2":
                    st = s_pool.tile([P, Cb], F32, tag="st", name="st")
                    nc.sync.dma_start(out=st[:], in_=s_d[:])
                    for i in range(IB):
                        w2t = w2_pool.tile([P, H], DT, tag=f"w2a_{i}",
                                           name=f"w2a_{i}")
                        nc.sync.dma_start(out=w2t[:], in_=w2rs[0][:, i, :])
                        w2ts[0].append(w2t)
                NH = H // NTOK  # 2
                # slot-B w2 streams in while the A blocks compute
                if nslots > 1:
                    for j in range(IB):
                        w2t = w2b_pool.tile([P, H], DT, tag=f"w2b_{j}",
                                            name=f"w2b_{j}")
                        nc.scalar.dma_start(out=w2t[:], in_=w2rs[1][:, j, :])
                        w2ts[1].append(w2t)
                # hld chunks: per tile, 2 token blocks (256 cols) per DMA
                cur_hld = {}
                for bi, (cb, s, t, cc) in enumerate(blocks):
                    pinned = t == 0 and hpins
                    if not pinned and (t, cc // (2 * P)) not in cur_hld:
                        w = tts[t][2]
                        j = cc // (2 * P)
                        cw = min(2 * P, w - j * 2 * P)
                        hld = hld_pool.tile([P, IB, 2 * P], DT, tag="hld",
                                            name=f"hld_{t}_{j}")
                        # sync queue: keeps hld clear of the w2b stream (scalar)
                        nc.sync.dma_start(
                            out=hld[:, :, :cw],
                            in_=ha_d[t, :, :, j * 2 * P: j * 2 * P + cw]
                            .rearrange("i p c -> p i c"))
                        cur_hld[(t, j)] = hld
                    yps = [
                        ps_pool.tile([P, NTOK], F32, tag="ps", name=f"yp_{cb}_{n}")
                        for n in range(NH)
                    ]
                    last = bi == len(blocks) - 1
                    k = (cc // P) % 2
                    if last:
                        # n-outer for the final block: n=0's epilogue overlaps
                        # n=1's matmuls, shrinking the tail
                        for n in range(NH):
                            for i in range(IB):
                                hsrc = (hpins[i][:, cc: cc + P] if pinned
                                        else cur_hld[(t, cc // (2 * P))]
                                        [:, i, k * P: (k + 1) * P])
                                nc.tensor.matmul(
                                    yps[n][:], hsrc,
                                    w2ts[s][i][:, n * NTOK: (n + 1) * NTOK],
                                    start=(i == 0), stop=(i == IB - 1),
                                )
                    else:
                        for i in range(IB):
                            hsrc = (hpins[i][:, cc: cc + P] if pinned
                                    else cur_hld[(t, cc // (2 * P))]
                                    [:, i, k * P: (k + 1) * P])
                            for n in range(NH):
                                nc.tensor.matmul(
                                    yps[n][:], hsrc,
                                    w2ts[s][i][:, n * NTOK: (n + 1) * NTOK],
                                    start=(i == 0), stop=(i == IB - 1),
                                )
                    for n in range(NH):
                        # final block: 256-wide chunks, scale-copy split across
                        # ACT and DVE, stores across both queues — drains the
                        # tail ~2x faster
                        nch = 2 if last else 1
                        for q in range(nch):
                            wq = NTOK // nch
                            ysb = y_pool.tile([P, wq], F32, tag="ysb",
                                              name=f"y_{cb}_{n}_{q}")
                            src = yps[n][:, q * wq: (q + 1) * wq]
                            if last and n == NH - 1 and q == 1:
                                nc.vector.tensor_scalar_mul(
                                    ysb[:], src, st[:, cb: cb + 1])
                                deng = nc.scalar
                            else:
                                nc.scalar.activation(
                                    ysb[:], src, AF.Copy,
                                    scale=st[:, cb: cb + 1])
                                deng = nc.sync
                            deng.dma_start(
                                out=y_d[
                                    rep * C + cb * P: rep * C + (cb + 1) * P,
                                    n * NTOK + q * wq: n * NTOK + (q + 1) * wq,
                                ],
                                in_=ysb[:],
                            )

    nc.compile()
    return nc


DTYPE = os.environ.get("MOE_DTYPE", "bf16")


def get_program(kA: int, kB: int) -> "bacc.Bacc":
    key = (kA, kB, DTYPE)
    if key not in _programs:
        _programs[key] = build_program(kA, kB, dtype=DTYPE)
    return _programs[key]


def _gate(x: np.ndarray, gate_w: np.ndarray):
    """Top-2 routing, mirroring the jax reference (softmax -> top_k ->
    renormalize). Uses jax for bit-compatible selection when available."""
    try:
        import jax
        import jax.numpy as jnp

        logits = jnp.asarray(x) @ jnp.asarray(gate_w)
        probs = jax.nn.softmax(logits, axis=-1)
        top_vals, top_idx = jax.lax.top_k(probs, TOP_K)
        top_vals = top_vals / jnp.sum(top_vals, axis=-1, keepdims=True)
        return np.asarray(top_vals), np.asarray(top_idx)
    except Exception:
        logits = x @ gate_w
        m = logits.max(-1, keepdims=True)
        p = np.exp(logits - m)
        p /= p.sum(-1, keepdims=True)
        top_idx = np.argsort(-p, axis=-1, kind="stable")[:, :TOP_K]
        top_vals = np.take_along_axis(p, top_idx, axis=-1)
        top_vals = top_vals / top_vals.sum(-1, keepdims=True)
        return top_vals, top_idx


def _route(x, gate_w):
    """Per-expert token index lists and routing weights."""
    top_vals, top_idx = _gate(x, gate_w)
    idxs, wts = [], []
    for e in range(E):
        sel = top_idx == e  # [T, K] bool
        mask = sel.any(axis=-1)
        idx_e = np.nonzero(mask)[0]
        w_e = np.where(sel[idx_e, 0], top_vals[idx_e, 0], top_vals[idx_e, 1])
        idxs.append(idx_e)
        wts.append(w_e.astype(np.float32))
    return idxs, wts


def _decompose(t_tiles, kA, kB):
    """Find per-expert (a_e, b_e) with sum(a)=sum(b)=8 and
    kA*a_e + kB*b_e >= t_e. Returns list of (a, b) or None."""
    nE = len(t_tiles)
    # DP over experts on (sum_a, sum_b)
    reach = {(0, 0): []}
    for e in range(nE):
        nxt = {}
        for (sa, sb), path in reach.items():
            for a in range(0, 9 - sa):
                # minimal b for this a
                need = t_tiles[e] - kA * a
                bmin = max(0, -(-need // kB))
                for b in range(bmin, 9 - sb):
                    k = (sa + a, sb + b)
                    if k not in nxt:
                        nxt[k] = path + [(a, b)]
        reach = nxt
        if not reach:
            return None
    return reach.get((8, 8))


def plan_dispatch(x, gate_w):
    """Balanced plan: per-core (expert_a, cols_a, expert_b, cols_b, scales).

    cols_* are padded token-id arrays (-1 = padding). Returns
    (kA, kB, per_core, idxs) or None if no balanced decomposition exists.
    """
    idxs, wts = _route(x, gate_w)
    t_tiles = [-(-max(len(ix), 1) // P) for ix in idxs]
    kT = -(-sum(t_tiles) // 8)
    found = None
    for kT_try in range(kT, kT + 3):
        for kA in range(kT_try - kT_try // 2, kT_try):
            kB = kT_try - kA
            ab = _decompose(t_tiles, kA, kB)
            if ab is not None:
                found = (kA, kB, ab)
                break
        if found:
            break
    if found is None:
        # single-slot fallback: one expert per core, capacity = max count
        kA, kB, ab = max(t_tiles), 0, [(1, 0)] * E
    else:
        kA, kB, ab = found

    # build pieces: expert e -> a_e chunks of kA*P slots, b_e of kB*P
    piecesA, piecesB = [], []
    for e in range(E):
        a, b = ab[e]
        padded = np.full(kA * P * a + kB * P * b, -1, np.int64)
        padded[: len(idxs[e])] = idxs[e]
        wpad = np.zeros(len(padded), np.float32)
        wpad[: len(idxs[e])] = wts[e]
        off = 0
        for _ in range(a):
            piecesA.append((e, padded[off: off + kA * P], wpad[off: off + kA * P]))
            off += kA * P
        for _ in range(b):
            piecesB.append((e, padded[off: off + kB * P], wpad[off: off + kB * P]))
            off += kB * P
    assert len(piecesA) == 8 and len(piecesB) in (0, 8)
    if kB == 0:
        empty = (np.empty(0, np.int64), np.zeros(0, np.float32))
        piecesB = [(piecesA[c][0],) + empty for c in range(8)]
    per_core = [piecesA[c] + piecesB[c] for c in range(8)]
    return kA, kB, per_core


def _block_w13(w):
    """[H, I] -> [P, IB, HB, P]: per-partition-contiguous stationary tiles."""
    return np.ascontiguousarray(
        w.reshape(H // P, P, I // P, P).transpose(1, 2, 0, 3))


def _npdt(dtype):
    if dtype == "bf16":
        import ml_dtypes
        return ml_dtypes.bfloat16
    return np.float32


def make_in_maps(x, w1, w3, w2, kA, kB, per_core, dtype=None):
    dtype = dtype or DTYPE
    npdt = _npdt(dtype)
    C = (kA + kB) * P
    in_maps = []
    w1b, w3b, w2b = {}, {}, {}

    def blocked(e):
        if e not in w1b:
            w1b[e] = _block_w13(np.asarray(w1[e], np.float32)).astype(npdt)
            w3b[e] = _block_w13(np.asarray(w3[e], np.float32)).astype(npdt)
            w2b[e] = np.ascontiguousarray(np.asarray(w2[e], np.float32)).astype(npdt)
        return w1b[e], w3b[e], w2b[e]

    for c in range(8):
        ea, cols_a, wts_a, eb, cols_b, wts_b = per_core[c]
        cols = np.concatenate([cols_a, cols_b])
        wcol = np.concatenate([wts_a, wts_b])
        x_pad = np.zeros((C, H), np.float32)
        real = cols >= 0
        x_pad[real] = x[cols[real]]
        w1a_, w3a_, w2a_ = blocked(ea)
        im = {
            "xt": np.ascontiguousarray(x_pad.T).astype(npdt),
            "w1a": w1a_, "w3a": w3a_, "w2a": w2a_,
            "st": np.ascontiguousarray(wcol.reshape(C // P, P).T),
        }
        if kB:
            im["w1b"], im["w3b"], im["w2b"] = blocked(eb)
        in_maps.append(im)
    return in_maps


def combine(results, per_core, T):
    out = np.zeros((T, H), np.float32)
    for c in range(8):
        ea, cols_a, _, eb, cols_b, _ = per_core[c]
        y = results[c]["y"]
        na = len(cols_a)
        for cols, rows in ((cols_a, y[:na]), (cols_b, y[na:])):
            real = cols >= 0
            out[cols[real]] += rows[real]
    return out


def kernel(hidden_states, gate_w, w1, w3, w2):
    B, S, Hh = hidden_states.shape
    assert Hh == H
    x = np.ascontiguousarray(hidden_states.reshape(-1, H), dtype=np.float32)
    T = x.shape[0]

    plan = plan_dispatch(x, gate_w)
    assert plan is not None, "no balanced decomposition; routing too skewed"
    kA, kB, per_core = plan
    nc = get_program(kA, kB)
    in_maps = make_in_maps(hidden_states.reshape(-1, H), w1, w3, w2,
                           kA, kB, per_core)
    res = run_bass_kernel_spmd(nc, in_maps, list(range(E)))
    out = combine(res.results, per_core, T)
    return out.reshape(B, S, H)


# revision 25
# speedup vs baseline: 1.0635x; 1.0201x over previous
"""MoE layer (top-2 routing, SwiGLU experts) for Trainium2, 8 NeuronCores.

Strategy: balanced expert parallelism. The gate (0.03% of FLOPs) and the
token dispatch/combine run on host. Tokens routed to each expert are
padded to 128-token tiles; the global tile count is split evenly across
the 8 cores (capacity = ceil(total_tiles/8) tiles/core instead of the
max-loaded expert), so every core does near-identical PE work. Each core
processes TWO token segments — slot A (kA tiles) and slot B (kB tiles)
with a GLOBAL static boundary — each segment belonging to one expert
whose weights are per-core inputs. Expert tile counts decompose into
kA/kB-sized pieces (e.g. 16=8+8, 17=9+8, 18=9+9), found by a small DP;
if no decomposition exists the kernel falls back to plain one-expert-
per-core with capacity = max expert count.

Each core runs the dense SwiGLU FFN in bf16 (PSUM accumulation is fp32;
tolerance is 2e-2, bf16 lands ~4e-3):

  phase 1:  h = silu(x @ w1) * (x @ w3)      (spilled to DRAM, token-tiled)
  phase 2:  y = (h @ w2) * route_weight      (tokens on partitions)

x is passed transposed ([H, CAP]) so phase 1 streams with tokens on the
free dimension and phase 2 uses h (token columns) as the matmul
stationary — no on-device transposes. w1/w3 are passed pre-blocked
[P, IB, HB, P] so every DMA moves >=1KB-contiguous per-partition
segments. w2 of slot A is prefetched during phase 1; w2 of slot B
streams during phase 2's A blocks, so no phase transition stalls.
"""

import os
import sys

for _p in ("/opt/trn_rl_repo", "/root/.axon_site/_ro/trn_rl_repo"):
    if os.path.isdir(_p) and _p not in sys.path:
        sys.path.insert(0, _p)

import numpy as np

import concourse.bass as bass  # noqa: F401  (bass must import before bacc)
import concourse.mybir as mybir
import concourse.tile as tile
from concourse import bacc
from concourse.bass_utils import run_bass_kernel_spmd

H = 1024
E = 8
I = 4096
TOP_K = 2
P = 128
NTOK = 512  # token tile width in phase 1 (PSUM bank = 512 fp32)
F32 = mybir.dt.float32
AF = mybir.ActivationFunctionType

_programs: dict = {}


def _tile_widths(span):
    """Split a column span into phase-1 tile widths (512s + remainder)."""
    out = [NTOK] * (span // NTOK)
    if span % NTOK:
        out.append(span % NTOK)
    return out


def build_program(kA: int, kB: int, repeat: int = 1, dtype: str = "bf16",
                  ph: str = "both", out_reps: int | None = None) -> "bacc.Bacc":
    """One-core SPMD program: SwiGLU FFN for two token segments.

    Slot A = kA*128 token columns of expert A, slot B = kB*128 columns of
    expert B (kB may be 0: single-slot program, used by the fallback
    path). repeat > 1 re-emits the whole computation (benchmarking aid);
    out_reps pads the output allocation so benchmark variants share
    identical output shapes (the axon tunnel dispatch latency keys on
    transfer sizes).
    """
    DT = {"f32": F32, "f32r": mybir.dt.float32r, "bf16": mybir.dt.bfloat16}[dtype]
    spans = [kA * P] + ([kB * P] if kB else [])
    C = sum(spans)
    Cb = C // P
    HB = H // P  # 8
    IB = I // P  # 32
    # per-slot phase-1 token tiles: (slot, start, width)
    tts = []
    off = 0
    for s, span in enumerate(spans):
        for w in _tile_widths(span):
            tts.append((s, off, w))
            off += w
    nslots = len(spans)

    nc = bacc.Bacc("TRN2", target_bir_lowering=False, debug=False, num_devices=8)
    x_d = nc.dram_tensor("xt", [H, C], DT, kind="ExternalInput")
    w1_ds = [nc.dram_tensor(f"w1{'ab'[s]}", [P, IB, HB, P], DT, kind="ExternalInput")
             for s in range(nslots)]
    w3_ds = [nc.dram_tensor(f"w3{'ab'[s]}", [P, IB, HB, P], DT, kind="ExternalInput")
             for s in range(nslots)]
    w2_ds = [nc.dram_tensor(f"w2{'ab'[s]}", [I, H], DT, kind="ExternalInput")
             for s in range(nslots)]
    s_d = nc.dram_tensor("st", [P, Cb], F32, kind="ExternalInput")
    out_reps = max(out_reps or repeat, repeat)
    y_d = nc.dram_tensor("y", [out_reps * C, H], F32, kind="ExternalOutput")
    # h_act scratch, token-tile-major so phase-1 writes land as one wide
    # [P, <=512] store and phase-2 reads come in >=256B segments
    ha_d = nc.dram_tensor("hact", [len(tts), IB, P, NTOK], DT)

    xr = x_d.rearrange("(h p) c -> h p c", p=P)  # [8, 128, C]
    w2rs = [w2_d.rearrange("(i p) n -> p i n", p=P) for w2_d in w2_ds]  # [128,32,1024]

    # phase-2 token blocks: (block, slot, tile, col-within-tile)
    blocks = []
    for t, (s, st0, w) in enumerate(tts):
        for j in range(w // P):
            blocks.append((st0 // P + j, s, t, j * P))

    # phase-1 matmul groups: per slot, chunks of <=3 tiles (6 PSUM banks)
    slot_groups = []
    for s in range(nslots):
        stile = [t for t in range(len(tts)) if tts[t][0] == s]
        for g in range(0, len(stile), 3):
            slot_groups.append((s, stile[g: g + 3]))
    # i=0 runs tile 0 alone first: its x chunk lands earliest, so the PE
    # starts while the rest of x is still in flight
    slot_groups_i0 = slot_groups
    if len(slot_groups[0][1]) > 1:
        s0, g0 = slot_groups[0]
        slot_groups_i0 = [(s0, g0[:1]), (s0, g0[1:])] + slot_groups[1:]

    with tile.TileContext(nc) as tc:
     # program-lifetime pool for each rep's first x chunk + slot-A w13[0]:
     # their loads overlap the previous rep's phase 2 (the per-rep pools'
     # regions only free at rep end, which would serialize these loads)
     with tc.tile_pool(name="warm", bufs=1) as warm_pool:
      for rep in range(repeat):
        # pools are scoped to phases so phase-2's w2b stack reuses the SBUF
        # region freed by phase-1's x tiles (both resident would overflow)
        with (
            tc.tile_pool(name=f"ps1{rep}", bufs=8, space="PSUM") as ps_pool,
            tc.tile_pool(name=f"w2{rep}", bufs=1) as w2_pool,
            tc.tile_pool(name=f"sc{rep}", bufs=1) as s_pool,
            tc.tile_pool(name=f"hp{rep}", bufs=1) as hp_pool,
        ):
          with (
            tc.tile_pool(name=f"xt{rep}", bufs=1) as xt_pool,
            tc.tile_pool(name=f"w13{rep}", bufs=8) as w13_pool,
            tc.tile_pool(name=f"tmp{rep}", bufs=4) as tmp_pool,
            tc.tile_pool(name=f"hst{rep}", bufs=4) as hst_pool,
          ):
            # ---------------- phase 1: h = silu(x@w1) * (x@w3) ----------------
            w2ts = {s: [] for s in range(nslots)}
            hpins = {}
            if ph in ("both", "p1"):
                # issue order tracks the PE's critical path: slot-A w13[i=0],
                # then x in three column chunks (first A tile, rest of A, B)
                # so the first matmuls fire as early as possible
                w13ts = {}

                def load_w13(i, slots=None):
                    tl = w13ts.setdefault(i, {})
                    for s in (range(nslots) if slots is None else slots):
                        if i == 0 and s == 0:
                            # warm pool: rep r+1's first weights load during
                            # rep r's phase 2 (per-rep pool regions would
                            # serialize this load until rep end)
                            pool, t1, t3 = warm_pool, "w13w1", "w13w3"
                        else:
                            pool, t1, t3 = w13_pool, "w13", "w13"
                        w1t = pool.tile([P, HB, P], DT, tag=t1,
                                        name=f"w1_{'ab'[s]}{i}_{rep}")
                        w3t = pool.tile([P, HB, P], DT, tag=t3,
                                        name=f"w3_{'ab'[s]}{i}_{rep}")
                        nc.sync.dma_start(out=w1t[:], in_=w1_ds[s][:, i])
                        nc.sync.dma_start(out=w3t[:], in_=w3_ds[s][:, i])
                        tl[s] = (w1t, w3t)

                load_w13(0, [0])
                W0 = min(NTOK, spans[0])  # first A tile
                W1 = spans[0]             # A/B boundary
                bnds = [0, W0, W1, C]
                xch = [[None] * 3 for _ in range(HB)]
                for ci in range(3):
                    lo, hi = bnds[ci], bnds[ci + 1]
                    if hi <= lo:
                        continue
                    for h in range(HB):
                        # alternate queues so early x tiles arrive twice as fast
                        eng = nc.scalar if h % 2 == 0 else nc.sync
                        pool = warm_pool if ci == 0 else xt_pool
                        xc = pool.tile([P, hi - lo], DT, tag=f"x{h}_{ci}",
                                       name=f"x{h}_{ci}_{rep}")
                        eng.dma_start(out=xc[:], in_=xr[h, :, lo:hi])
                        xch[h][ci] = xc
                    if ci == 0 and nslots > 1:
                        load_w13(0, [1])

                def xs(t, h):
                    """x slice [P, w] for token tile t, h-block h."""
                    _, st0, w = tts[t]
                    for ci in range(3):
                        if st0 < bnds[ci + 1]:
                            return xch[h][ci][:, st0 - bnds[ci]:
                                              st0 - bnds[ci] + w]
                    raise AssertionError

                load_w13(1)
                if ph == "both":
                    st = s_pool.tile([P, Cb], F32, tag="st", name="st")
                    nc.scalar.dma_start(out=st[:], in_=s_d[:])

                for i in range(IB):
                    if i not in w13ts:
                        load_w13(i)
                    # phase-2 slot-A weight prefetch, 2 tiles per iteration
                    if ph == "both" and i < IB // 2:
                        for j in (2 * i, 2 * i + 1):
                            w2t = w2_pool.tile([P, H], DT, tag=f"w2a_{j}",
                                               name=f"w2a_{j}")
                            nc.scalar.dma_start(out=w2t[:], in_=w2rs[0][:, j, :])
                            w2ts[0].append(w2t)
                    # per slot: w1 matmuls for all its tiles, then w3 — each
                    # stationary serves the whole slot span before switching.
                    # Slots wider than 3 tiles are chunked (PSUM = 8 banks).
                    for s, stile in (slot_groups_i0 if i == 0 else slot_groups):
                        w1t, w3t = w13ts[i][s]
                        p1s, p3s = {}, {}
                        for t in stile:
                            w = tts[t][2]
                            p1s[t] = ps_pool.tile([P, NTOK], F32, tag="ps",
                                                  name=f"p1_{i}_{t}")
                            p3s[t] = ps_pool.tile([P, NTOK], F32, tag="ps",
                                                  name=f"p3_{i}_{t}")
                        for h in range(HB):
                            for t in stile:
                                nc.tensor.matmul(
                                    p1s[t][:, : tts[t][2]], w1t[:, h, :], xs(t, h),
                                    start=(h == 0), stop=(h == HB - 1),
                                )
                        for h in range(HB):
                            for t in stile:
                                nc.tensor.matmul(
                                    p3s[t][:, : tts[t][2]], w3t[:, h, :], xs(t, h),
                                    start=(h == 0), stop=(h == HB - 1),
                                )
                        for t in stile:
                            w, p1, p3 = tts[t][2], p1s[t], p3s[t]
                            tmp = tmp_pool.tile([P, NTOK], F32, tag="tmp",
                                                name=f"tmp_{i}_{t}")
                            nc.scalar.activation(tmp[:, :w], p1[:, :w], AF.Silu)
                            if t == 0 and ph == "both":
                                # token tile 0's h stays pinned in SBUF: phase 2
                                # starts on it with no DMA dependency, hiding
                                # the hld loads for later tiles
                                hst = hp_pool.tile([P, NTOK], DT, tag=f"hp{i}",
                                                   name=f"hp_{i}")
                                hpins[i] = hst
                                nc.vector.tensor_mul(hst[:, :w], tmp[:, :w], p3[:, :w])
                            else:
                                hst = hst_pool.tile([P, NTOK], DT, tag="hst",
                                                    name=f"h_{i}_{t}")
                                nc.vector.tensor_mul(hst[:, :w], tmp[:, :w], p3[:, :w])
                                nc.sync.dma_start(out=ha_d[t, i, :, :w],
                                                  in_=hst[:, :w])

          # ---------------- phase 2: y = (h @ w2) * s ----------------
          with (
            tc.tile_pool(name=f"w2b{rep}", bufs=1) as w2b_pool,
            tc.tile_pool(name=f"hld{rep}", bufs=2) as hld_pool,
            tc.tile_pool(name=f"ysb{rep}", bufs=3) as y_pool,
          ):
            if ph in ("both", "p2"):
                if ph == "p2":
                    st = s_pool.tile([P, Cb], F32, tag="st", name="st")
                    nc.sync.dma_start(out=st[:], in_=s_d[:])
                    for i in range(IB):
                        w2t = w2_pool.tile([P, H], DT, tag=f"w2a_{i}",
                                           name=f"w2a_{i}")
                        nc.sync.dma_start(out=w2t[:], in_=w2rs[0][:, i, :])
                        w2ts[0].append(w2t)
                NH = H // NTOK  # 2
                # slot-B w2 streams in while the A blocks compute
                if nslots > 1:
                    for j in range(IB):
                        w2t = w2b_pool.tile([P, H], DT, tag=f"w2b_{j}",
                                            name=f"w2b_{j}")
                        nc.scalar.dma_start(out=w2t[:], in_=w2rs[1][:, j, :])
                        w2ts[1].append(w2t)
                # hld chunks: per tile, 2 token blocks (256 cols) per DMA
                cur_hld = {}
                for bi, (cb, s, t, cc) in enumerate(blocks):
                    pinned = t == 0 and hpins
                    if not pinned and (t, cc // (2 * P)) not in cur_hld:
                        w = tts[t][2]
                        j = cc // (2 * P)
                        cw = min(2 * P, w - j * 2 * P)
                        hld = hld_pool.tile([P, IB, 2 * P], DT, tag="hld",
                                            name=f"hld_{t}_{j}")
                        # sync queue: keeps hld clear of the w2b stream (scalar)
                        nc.sync.dma_start(
                            out=hld[:, :, :cw],
                            in_=ha_d[t, :, :, j * 2 * P: j * 2 * P + cw]
                            .rearrange("i p c -> p i c"))
                        cur_hld[(t, j)] = hld
                    yps = [
                        ps_pool.tile([P, NTOK], F32, tag="ps", name=f"yp_{cb}_{n}")
                        for n in range(NH)
                    ]
                    last = bi == len(blocks) - 1
                    k = (cc // P) % 2
                    if last:
                        # n-outer for the final block: n=0's epilogue overlaps
                        # n=1's matmuls, shrinking the tail
                        for n in range(NH):
                            for i in range(IB):
                                hsrc = (hpins[i][:, cc: cc + P] if pinned
                                        else cur_hld[(t, cc // (2 * P))]
                                        [:, i, k * P: (k + 1) * P])
                                nc.tensor.matmul(
                                    yps[n][:], hsrc,
                                    w2ts[s][i][:, n * NTOK: (n + 1) * NTOK],
                                    start=(i == 0), stop=(i == IB - 1),
                                )
                    else:
                        for i in range(IB):
                            hsrc = (hpins[i][:, cc: cc + P] if pinned
                                    else cur_hld[(t, cc // (2 * P))]
                                    [:, i, k * P: (k + 1) * P])
                            for n in range(NH):
                                nc.tensor.matmul(
                                    yps[n][:], hsrc,
                                    w2ts[s][i][:, n * NTOK: (n + 1) * NTOK],
                                    start=(i == 0), stop=(i == IB - 1),
                                )
                    for n in range(NH):
                        # 256-wide chunks keep the ysb pool small (frees SBUF
                        # for the warm pool); the final block also splits the
                        # scale-copy across ACT and DVE and the stores across
                        # both queues, draining the tail ~2x faster
                        nch = 2
                        for q in range(nch):
                            wq = NTOK // nch
                            ysb = y_pool.tile([P, wq], F32, tag="ysb",
                                              name=f"y_{cb}_{n}_{q}")
                            src = yps[n][:, q * wq: (q + 1) * wq]
                            if last and n == NH - 1 and q == 1:
                                nc.vector.tensor_scalar_mul(
                                    ysb[:], src, st[:, cb: cb + 1])
                                deng = nc.scalar
                            else:
                                nc.scalar.activation(
                                    ysb[:], src, AF.Copy,
                                    scale=st[:, cb: cb + 1])
                                deng = nc.sync
                            deng.dma_start(
                                out=y_d[
                                    rep * C + cb * P: rep * C + (cb + 1) * P,
                                    n * NTOK + q * wq: n * NTOK + (q + 1) * wq,
                                ],
                                in_=ysb[:],
                            )

    nc.compile()
    return nc


DTYPE = os.environ.get("MOE_DTYPE", "bf16")


def get_program(kA: int, kB: int) -> "bacc.Bacc":
    key = (kA, kB, DTYPE)
    if key not in _programs:
        _programs[key] = build_program(kA, kB, dtype=DTYPE)
    return _programs[key]


def _gate(x: np.ndarray, gate_w: np.ndarray):
    """Top-2 routing, mirroring the jax reference (softmax -> top_k ->
    renormalize). Uses jax for bit-compatible selection when available."""
    try:
        import jax
        import jax.numpy as jnp

        logits = jnp.asarray(x) @ jnp.asarray(gate_w)
        probs = jax.nn.softmax(logits, axis=-1)
        top_vals, top_idx = jax.lax.top_k(probs, TOP_K)
        top_vals = top_vals / jnp.sum(top_vals, axis=-1, keepdims=True)
        return np.asarray(top_vals), np.asarray(top_idx)
    except Exception:
        logits = x @ gate_w
        m = logits.max(-1, keepdims=True)
        p = np.exp(logits - m)
        p /= p.sum(-1, keepdims=True)
        top_idx = np.argsort(-p, axis=-1, kind="stable")[:, :TOP_K]
        top_vals = np.take_along_axis(p, top_idx, axis=-1)
        top_vals = top_vals / top_vals.sum(-1, keepdims=True)
        return top_vals, top_idx


def _route(x, gate_w):
    """Per-expert token index lists and routing weights."""
    top_vals, top_idx = _gate(x, gate_w)
    idxs, wts = [], []
    for e in range(E):
        sel = top_idx == e  # [T, K] bool
        mask = sel.any(axis=-1)
        idx_e = np.nonzero(mask)[0]
        w_e = np.where(sel[idx_e, 0], top_vals[idx_e, 0], top_vals[idx_e, 1])
        idxs.append(idx_e)
        wts.append(w_e.astype(np.float32))
    return idxs, wts


def _decompose(t_tiles, kA, kB):
    """Find per-expert (a_e, b_e) with sum(a)=sum(b)=8 and
    kA*a_e + kB*b_e >= t_e. Returns list of (a, b) or None."""
    nE = len(t_tiles)
    # DP over experts on (sum_a, sum_b)
    reach = {(0, 0): []}
    for e in range(nE):
        nxt = {}
        for (sa, sb), path in reach.items():
            for a in range(0, 9 - sa):
                # minimal b for this a
                need = t_tiles[e] - kA * a
                bmin = max(0, -(-need // kB))
                for b in range(bmin, 9 - sb):
                    k = (sa + a, sb + b)
                    if k not in nxt:
                        nxt[k] = path + [(a, b)]
        reach = nxt
        if not reach:
            return None
    return reach.get((8, 8))


def plan_dispatch(x, gate_w):
    """Balanced plan: per-core (expert_a, cols_a, expert_b, cols_b, scales).

    cols_* are padded token-id arrays (-1 = padding). Returns
    (kA, kB, per_core, idxs) or None if no balanced decomposition exists.
    """
    idxs, wts = _route(x, gate_w)
    t_tiles = [-(-max(len(ix), 1) // P) for ix in idxs]
    kT = -(-sum(t_tiles) // 8)
    found = None
    for kT_try in range(kT, kT + 3):
        for kA in range(kT_try - kT_try // 2, kT_try):
            kB = kT_try - kA
            ab = _decompose(t_tiles, kA, kB)
            if ab is not None:
                found = (kA, kB, ab)
                break
        if found:
            break
    if found is None:
        # single-slot fallback: one expert per core, capacity = max count
        kA, kB, ab = max(t_tiles), 0, [(1, 0)] * E
    else:
        kA, kB, ab = found

    # build pieces: expert e -> a_e chunks of kA*P slots, b_e of kB*P
    piecesA, piecesB = [], []
    for e in range(E):
        a, b = ab[e]
        padded = np.full(kA * P * a + kB * P * b, -1, np.int64)
        padded[: len(idxs[e])] = idxs[e]
        wpad = np.zeros(len(padded), np.float32)
        wpad[: len(idxs[e])] = wts[e]
        off = 0
        for _ in range(a):
            piecesA.append((e, padded[off: off + kA * P], wpad[off: off + kA * P]))
            off += kA * P
        for _ in range(b):
            piecesB.append((e, padded[off: off + kB * P], wpad[off: off + kB * P]))
            off += kB * P
    assert len(piecesA) == 8 and len(piecesB) in (0, 8)
    if kB == 0:
        empty = (np.empty(0, np.int64), np.zeros(0, np.float32))
        piecesB = [(piecesA[c][0],) + empty for c in range(8)]
    per_core = [piecesA[c] + piecesB[c] for c in range(8)]
    return kA, kB, per_core


def _block_w13(w):
    """[H, I] -> [P, IB, HB, P]: per-partition-contiguous stationary tiles."""
    return np.ascontiguousarray(
        w.reshape(H // P, P, I // P, P).transpose(1, 2, 0, 3))


def _npdt(dtype):
    if dtype == "bf16":
        import ml_dtypes
        return ml_dtypes.bfloat16
    return np.float32


def make_in_maps(x, w1, w3, w2, kA, kB, per_core, dtype=None):
    dtype = dtype or DTYPE
    npdt = _npdt(dtype)
    C = (kA + kB) * P
    in_maps = []
    w1b, w3b, w2b = {}, {}, {}

    def blocked(e):
        if e not in w1b:
            w1b[e] = _block_w13(np.asarray(w1[e], np.float32)).astype(npdt)
            w3b[e] = _block_w13(np.asarray(w3[e], np.float32)).astype(npdt)
            w2b[e] = np.ascontiguousarray(np.asarray(w2[e], np.float32)).astype(npdt)
        return w1b[e], w3b[e], w2b[e]

    for c in range(8):
        ea, cols_a, wts_a, eb, cols_b, wts_b = per_core[c]
        cols = np.concatenate([cols_a, cols_b])
        wcol = np.concatenate([wts_a, wts_b])
        x_pad = np.zeros((C, H), np.float32)
        real = cols >= 0
        x_pad[real] = x[cols[real]]
        w1a_, w3a_, w2a_ = blocked(ea)
        im = {
            "xt": np.ascontiguousarray(x_pad.T).astype(npdt),
            "w1a": w1a_, "w3a": w3a_, "w2a": w2a_,
            "st": np.ascontiguousarray(wcol.reshape(C // P, P).T),
        }
        if kB:
            im["w1b"], im["w3b"], im["w2b"] = blocked(eb)
        in_maps.append(im)
    return in_maps


def combine(results, per_core, T):
    out = np.zeros((T, H), np.float32)
    for c in range(8):
        ea, cols_a, _, eb, cols_b, _ = per_core[c]
        y = results[c]["y"]
        na = len(cols_a)
        for cols, rows in ((cols_a, y[:na]), (cols_b, y[na:])):
            real = cols >= 0
            out[cols[real]] += rows[real]
    return out


def kernel(hidden_states, gate_w, w1, w3, w2):
    B, S, Hh = hidden_states.shape
    assert Hh == H
    x = np.ascontiguousarray(hidden_states.reshape(-1, H), dtype=np.float32)
    T = x.shape[0]

    plan = plan_dispatch(x, gate_w)
    assert plan is not None, "no balanced decomposition; routing too skewed"
    kA, kB, per_core = plan
    nc = get_program(kA, kB)
    in_maps = make_in_maps(hidden_states.reshape(-1, H), w1, w3, w2,
                           kA, kB, per_core)
    res = run_bass_kernel_spmd(nc, in_maps, list(range(E)))
    out = combine(res.results, per_core, T)
    return out.reshape(B, S, H)
